# revision 32
# baseline (speedup 1.0000x reference)
# Trainium2 Bass kernel for Mistral-style sliding-window GQA attention.
#
# Problem: hidden [2,1024,4096], 32 q-heads / 8 kv-heads, head_dim 128,
# RoPE (neox), causal + sliding-window(512) attention, out proj.
#
# Sharding: tensor-parallel over heads across 8 cores. Core c owns q-heads
# [4c..4c+3] and kv-head c (wq cols 512c:512c+512, wk/wv cols 128c:+128).
# Each core computes its heads' attention output in TRANSPOSED layout
# [feat, tok]; a per-batch AllGather over the 8 cores concatenates the
# feature (partition) axis to give the full [4096, 1024] attn output of that
# batch on every core, and each core then applies its column shard of wo
# ([4096, 512]) to produce out[:, 512c:512c+512]. The host concatenates the
# 8 column shards. Each AllGather overlaps the other batch's projections /
# attention / out-projection.
#
# All matmuls run in bf16 (fp32 PSUM accumulation); softmax math in fp32.
#
# Layout trick: everything is computed transposed ([feature, token]) so that
# every matmul's contraction operand is already partition-major:
#   QT = wq.T @ hid     via matmul(lhsT=wq_chunk,  rhs=hidT_chunk)
#   KT = wk.T @ hid     via matmul(lhsT=wk_chunk,  rhs=hidT_chunk)
#   VT = wv.T @ hid     via matmul(lhsT=wv_chunk,  rhs=hidT_chunk)
#   V  = VT.T           via 16 PE transposes (V needed k-major for O^T)
#   ST = K_j^T Q        via matmul(lhsT=KT_j,      rhs=QT_span)   [k, q]
#   l  = 1^T A          via matmul(lhsT=ones,      rhs=attnT)     [1, q]
#   OT = V_j^T A        via matmul(lhsT=V_j,       rhs=attnT)     [d, q]
#   out= ag^T @ wo      via matmul(lhsT=ag_chunk,  rhs=wo_chunk)  [tok, oc]
#
# Masking: causal+window only bites on the diagonal k==q tile and (for
# full-width spans) the window-edge tile. Instead of adding -1e30 via a
# second PE matmul pass (which doubles the score-stream cost), we exp the
# raw scores and multiply the two boundary 128x128 subtiles by 0/1 masks on
# the DVE (cheap; scores are O(10) so exp never overflows).
#
# PSUM bank-B init: the 5-tile k-span accumulation into the second psum
# bank starts with k-tile j=4, whose q-span exactly covers the bank
# (cols 512:1024), so start=True there initializes the bank and every other
# tile accumulates -- no dummy zero matmuls.
#
# Scheduling: per-batch AllGather triggers sit on the gpsimd queue right
# after that batch's attention-output writes, so they fire as early as the
# data allows. tc.no_sync_barrier() fences before each out-projection stop
# the Tile scheduler from hoisting collective-dependent ag loads above the
# earlier hid loads on the same DMA engine (which serialized a whole
# collective into the PE critical path in the previous version).

import functools

import numpy as np
import ml_dtypes

BF16 = ml_dtypes.bfloat16

B, S, HID = 2, 1024, 4096
T = B * S                     # 2048 flattened tokens
NCORES = 8
D = 128                       # head dim
QH = 4                        # q heads per core
QF = QH * D                   # 512 q features per core
HC = HID // 128               # 32 hidden-dim chunks
NT = 4                        # 512-token chunks
NJ = S // 128                 # 8 k-tiles per batch
WINDOW = 512
NB = WINDOW // 128 + 1        # 5-tile k-span per q-tile
SPAN = NB * 128               # 640
OUTC = HID // NCORES          # 512 out columns per core
SCALE = D ** -0.5


def _build():
    import concourse.mybir as mybir
    import concourse.tile as tile
    from concourse import bacc
    from concourse.masks import make_identity

    f32, bf16 = mybir.dt.float32, mybir.dt.bfloat16
    AF = mybir.ActivationFunctionType
    ALU = mybir.AluOpType

    nc = bacc.Bacc(
        "TRN2", target_bir_lowering=False, debug=False, num_devices=NCORES
    )

    hidT = nc.dram_tensor("hidT", [128, HC * T], bf16, kind="ExternalInput")
    wq = nc.dram_tensor("wq", [128, HC * QF], bf16, kind="ExternalInput")
    wk = nc.dram_tensor("wk", [128, HC * D], bf16, kind="ExternalInput")
    wv = nc.dram_tensor("wv", [128, HC * D], bf16, kind="ExternalInput")
    wo = nc.dram_tensor("wo", [128, HC * OUTC], bf16, kind="ExternalInput")
    cosT = nc.dram_tensor("cosT", [128, T], f32, kind="ExternalInput")
    sinT = nc.dram_tensor("sinT", [128, T], f32, kind="ExternalInput")
    # [128, 256]: cols 0:128 = diag causal 0/1 mask, 128:256 = window edge
    m01 = nc.dram_tensor("m01", [128, 256], bf16, kind="ExternalInput")
    out = nc.dram_tensor("out", [T, OUTC], bf16, kind="ExternalOutput")

    with tile.TileContext(nc) as tc:
        with (
            tc.tile_pool(name="ps", bufs=4, space="PSUM") as psp,
            tc.tile_pool(name="consts", bufs=1) as sbp,
            tc.tile_pool(name="hidp", bufs=8) as hidp,
            tc.tile_pool(name="ropep", bufs=2) as ropep,
            tc.tile_pool(name="attnp", bufs=4) as attnp,
            tc.tile_pool(name="miscp", bufs=2) as miscp,
            tc.tile_pool(name="agp", bufs=20) as agp,
            tc.tile_pool(name="dram", bufs=1, space="DRAM") as dramp,
        ):
            hidT_r = hidT[:, :].rearrange("p (h t) -> p h t", h=HC)

            def load_hid(n, h2, eng):
                # one DMA covers h-chunks 2*h2 and 2*h2+1 (fewer descriptors,
                # fewer DMA semaphore waits on the PE)
                ht = hidp.tile([128, 1024], bf16, tag="hid", name=f"ht{n}_{h2}")
                eng.dma_start(
                    ht[:].rearrange("p (a t) -> p a t", a=2),
                    hidT_r[:, 2 * h2:2 * h2 + 2, n * 512:(n + 1) * 512],
                )
                return ht

            # startup-critical loads first: the h=0..1 chunks of wq/wk/wv
            # feed the very first matmuls; everything else queues behind in
            # consumption order on the sync DGE queue.
            wq_sb = sbp.tile([128, HC * QF], bf16, name="wq_sb")
            wk_sb = sbp.tile([128, HC * D], bf16, name="wk_sb")
            wv_sb = sbp.tile([128, HC * D], bf16, name="wv_sb")
            nc.sync.dma_start(wq_sb[:, 0:1024], wq[:, 0:1024])
            nc.sync.dma_start(wk_sb[:, 0:1024], wk[:, 0:1024])
            nc.sync.dma_start(wv_sb[:, 0:1024], wv[:, 0:1024])
            pre0 = [load_hid(0, h2, nc.scalar) for h2 in range(6)]
            # remaining weight pieces, interleaved roughly in the order the
            # A0 matmuls consume them (wq piece p feeds h=2p..2p+1; wk/wv
            # piece p feeds h=8p..8p+7)
            wsplit = []
            for p in range(1, 16):
                wsplit.append((wq_sb, wq, p))
                if p in (3, 7, 11):
                    kp = (p + 1) // 4
                    wsplit.append((wk_sb, wk, kp))
                    wsplit.append((wv_sb, wv, kp))
            for dst, src, p in wsplit:
                nc.sync.dma_start(
                    dst[:, p * 1024:(p + 1) * 1024],
                    src[:, p * 1024:(p + 1) * 1024],
                )
            cos_sb = sbp.tile([128, T], f32, name="cos_sb")
            sin_sb = sbp.tile([128, T], f32, name="sin_sb")
            for i in range(4):
                nc.gpsimd.dma_start(
                    cos_sb[:, i * 512:(i + 1) * 512],
                    cosT[:, i * 512:(i + 1) * 512],
                )
                nc.gpsimd.dma_start(
                    sin_sb[:, i * 512:(i + 1) * 512],
                    sinT[:, i * 512:(i + 1) * 512],
                )

            QT_sb = sbp.tile([128, QH * T], bf16, name="QT_sb")
            KT_sb = sbp.tile([128, T], bf16, name="KT_sb")
            VT_sb = sbp.tile([128, T], bf16, name="VT_sb")
            V_sb = sbp.tile([128, T], bf16, name="V_sb")

            # two AllGathers per batch (head-pair split), triggered as soon
            # as that batch's attention outputs land; halving the payload
            # halves each collective's completion latency, so the out
            # projection ungates sooner. ag rows stay head-major per pair.
            attn_local = [
                [dramp.tile([2 * D, S], bf16, name=f"attn_local{b}_{hp}")
                 for hp in range(2)]
                for b in range(B)
            ]
            ag_out = [
                [dramp.tile([NCORES * 2 * D, S], bf16, name=f"ag_out{b}_{hp}",
                            addr_space="Shared")
                 for hp in range(2)]
                for b in range(B)
            ]

            def rope(src_ps, stg_sw, dst_sb, col0, tok0):
                # neox rotate-half:
                #   out = x*cos + swap_halves(x)*sin_signed
                # (sin table rows 0:64 arrive pre-negated from the host)
                # The unswapped term reads the psum accumulator directly on
                # DVE; only the half-swap needs an ACT staging copy.
                c = cos_sb[:, tok0:tok0 + 512]
                sg = sin_sb[:, tok0:tok0 + 512]
                t1 = ropep.tile([128, 512], f32, tag="rt1", name="t1")
                t2 = ropep.tile([128, 512], f32, tag="rt2", name="t2")
                nc.vector.tensor_tensor(t1[:], src_ps, c, ALU.mult)
                nc.vector.tensor_tensor(t2[:], stg_sw[:], sg, ALU.mult)
                nc.vector.tensor_tensor(
                    dst_sb[:, col0:col0 + 512], t1[:], t2[:], ALU.add
                )

            def rope_stage(src_ps, tag):
                # half-swapped psum -> sbuf copy on ACT
                stg_sw = ropep.tile([128, 512], f32, tag="stgsw", bufs=2,
                                    name=f"stgsw{tag}")
                nc.scalar.copy(stg_sw[0:64, :], src_ps[64:128, :])
                nc.scalar.copy(stg_sw[64:128, :], src_ps[0:64, :])
                return stg_sw

            # small constants needed by phase B (tiny DMAs / on-chip init)
            mask_sb = sbp.tile([128, 256], bf16, name="mask_sb")
            nc.sync.dma_start(mask_sb[:], m01[:, :])
            ones_sb = sbp.tile([128, 1], bf16, name="ones_sb")
            nc.vector.memset(ones_sb[:], 1.0)
            ident_sb = sbp.tile([128, 128], bf16, name="ident_sb")
            make_identity(nc, ident_sb[:])
            wo_sb = sbp.tile([128, HC * OUTC], bf16, name="wo_sb")

            # ---- V = VT.T via PE transposes (folded per 512-tok chunk) ----
            def v_transpose(tt_lo, tt_hi):
                for tt in range(tt_lo, tt_hi):
                    trp = psp.tile([128, 128], bf16, tag="big", name=f"tr{tt}")
                    nc.tensor.transpose(
                        trp[:], VT_sb[:, tt * 128:(tt + 1) * 128], ident_sb[:]
                    )
                    nc.vector.tensor_copy(V_sb[:, tt * 128:(tt + 1) * 128], trp[:])

            # ---- phase A: projections, all transposed, weight-stationary ----
            def phase_a(n, eng, pre_tiles=(), trs=()):
                qa = psp.tile([128, 1024], f32, tag="big", name=f"qa{n}")
                qb = psp.tile([128, 1024], f32, tag="big", name=f"qb{n}")
                kvt = psp.tile([128, 1024], f32, tag="big", name=f"kvt{n}")
                G = 4
                for hg in range(0, HC, G):
                    # earlier chunk's V transposes ride along in the matmul
                    # stream (PE work, no separate phase-boundary bubble)
                    gi = hg // G
                    if 1 <= gi <= len(trs):
                        v_transpose(trs[gi - 1], trs[gi - 1] + 1)
                    pairs = []
                    for h2 in range(hg // 2, hg // 2 + G // 2):
                        if h2 < len(pre_tiles):
                            pairs.append(pre_tiles[h2])
                        else:
                            pairs.append(load_hid(n, h2, eng))
                    hts = [
                        pairs[k // 2][:, (k % 2) * 512:(k % 2) * 512 + 512]
                        for k in range(G)
                    ]
                    # run-length-4 per psum bank: keeps the PE from
                    # micro-idling on per-matmul psum-bank switches
                    for m in range(QH):
                        ps = qa if m < 2 else qb
                        o = (m % 2) * 512
                        for k, h in enumerate(range(hg, hg + G)):
                            nc.tensor.matmul(
                                ps[:, o:o + 512],
                                wq_sb[:, (h * QH + m) * 128:(h * QH + m + 1) * 128],
                                hts[k],
                                start=(h == 0), stop=(h == HC - 1),
                            )
                    for k, h in enumerate(range(hg, hg + G)):
                        nc.tensor.matmul(
                            kvt[:, 0:512],
                            wk_sb[:, h * 128:(h + 1) * 128],
                            hts[k],
                            start=(h == 0), stop=(h == HC - 1),
                        )
                    for k, h in enumerate(range(hg, hg + G)):
                        nc.tensor.matmul(
                            kvt[:, 512:1024],
                            wv_sb[:, h * 128:(h + 1) * 128],
                            hts[k],
                            start=(h == 0), stop=(h == HC - 1),
                        )
                # K first so phase B's first score matmuls unblock earliest
                sksw = rope_stage(kvt[:, 0:512], f"K{n}")
                nc.scalar.copy(VT_sb[:, n * 512:(n + 1) * 512], kvt[:, 512:1024])
                rope(kvt[:, 0:512], sksw, KT_sb, n * 512, n * 512)
                for m in range(QH):
                    ps = qa if m < 2 else qb
                    ph = ps[:, (m % 2) * 512:(m % 2) * 512 + 512]
                    sqsw = rope_stage(ph, f"q{n}_{m}")
                    rope(ph, sqsw, QT_sb, m * T + n * 512, n * 512)

            # ---- phase B: windowed attention + per-batch AllGathers ----
            # Natural k-tile order. start=True on j=0's segment of each
            # psum bank clears that bank's has_written bits once; every
            # later segment uses start=False -- each column's first writer
            # overwrites (bit clear) and subsequent ones accumulate, so no
            # zero-init dummies are needed.
            def acc_segments(j):
                # (lo, hi, start, stop) column segments for k-tile j
                w = min(NB, NJ - j) * 128
                lo, hi = 128 * j, 128 * j + w
                segs = []
                if lo < 512:
                    segs.append((lo, min(hi, 512), j == 0, j == 3))
                if hi > 512:
                    segs.append((max(lo, 512), hi, j == 0, j == NJ - 1))
                return segs

            def phase_b(b):
                for m in range(QH):
                    at_tiles = {}

                    def scores(j, m=m):
                        # scores^T tile [k, q-span]; raw scores, exp on ACT,
                        # then 0/1 boundary masks on DVE
                        w = min(NB, NJ - j) * 128
                        sc = psp.tile(
                            [128, 1024], f32, tag="big", name=f"sc{b}_{m}_{j}"
                        )
                        kslice = KT_sb[:, b * S + j * 128:b * S + (j + 1) * 128]
                        q0 = m * T + b * S + j * 128
                        for o in range(0, w, 512):
                            nw = min(512, w - o)
                            nc.tensor.matmul(
                                sc[:, o:o + nw], kslice,
                                QT_sb[:, q0 + o:q0 + o + nw],
                                start=True, stop=True,
                            )
                        at = attnp.tile(
                            [128, SPAN], bf16, tag="attn", bufs=4,
                            name=f"at{b}{m}{j}"
                        )
                        nc.scalar.activation(at[:, :w], sc[:, :w], AF.Exp)
                        # causal mask on the diagonal subtile; window-edge
                        # mask on the last subtile of full-width spans
                        nc.vector.tensor_tensor(
                            at[:, 0:128], at[:, 0:128], mask_sb[:, 0:128],
                            ALU.mult,
                        )
                        if w == SPAN:
                            nc.vector.tensor_tensor(
                                at[:, 512:640], at[:, 512:640],
                                mask_sb[:, 128:256], ALU.mult,
                            )
                        at_tiles[j] = at

                    # the two leading score tiles allocate their psum slots
                    # BEFORE l/o, so the head starts the instant the first
                    # slot frees from the previous phase's rope chain
                    scores(0)
                    scores(1)

                    l_ps = psp.tile([128, 1024], f32, tag="big", name=f"l{b}_{m}")
                    o_ps = psp.tile([128, 1024], f32, tag="big", name=f"o{b}_{m}")

                    def norm_half(hb, mm=m, bb=b, l_ps=l_ps, o_ps=o_ps):
                        # copy the stopped psum banks out immediately (o on
                        # ACT, l on DVE) so the slots free for the next
                        # head; the bcast/recip/mult normalize chain then
                        # runs entirely SBUF-side, off the PE critical path
                        c0 = hb * 512
                        o_sb = miscp.tile([128, 512], bf16, tag="ocp",
                                          name=f"o_sb{bb}{mm}{hb}")
                        nc.scalar.copy(o_sb[:], o_ps[:, c0:c0 + 512])
                        l_sb = miscp.tile([1, 512], f32, tag="lsb",
                                          name=f"l_sb{bb}{mm}{hb}")
                        nc.vector.tensor_copy(l_sb[:], l_ps[0:1, c0:c0 + 512])
                        bc = miscp.tile([128, 512], f32, tag="bcast",
                                        name=f"bc{bb}{mm}{hb}")
                        nc.gpsimd.partition_broadcast(bc[:], l_sb[:])
                        bcr = miscp.tile([128, 512], f32, tag="bcr",
                                         name=f"bcr{bb}{mm}{hb}")
                        nc.vector.reciprocal_approx_fast(bcr[:], bc[:])
                        oT = miscp.tile([128, 512], bf16, tag="osb",
                                        name=f"oT{bb}{mm}{hb}")
                        nc.vector.tensor_tensor(
                            oT[:], o_sb[:], bcr[:], ALU.mult
                        )
                        nc.gpsimd.dma_start(
                            attn_local[bb][mm // 2][(mm % 2) * 128:
                                                    (mm % 2) * 128 + 128,
                                                    c0:c0 + 512], oT[:]
                        )

                    def acc(j, which):
                        # span accumulation per psum bank (l: ones, oT: V_j)
                        at = at_tiles[j]
                        vslice = V_sb[:, (b * NJ + j) * 128:(b * NJ + j + 1) * 128]
                        for (lo, hi, st, sp) in acc_segments(j):
                            a = at[:, lo - 128 * j: hi - 128 * j]
                            if which == "l":
                                nc.tensor.matmul(
                                    l_ps[0:1, lo:hi], ones_sb[:], a,
                                    start=st, stop=sp,
                                )
                            else:
                                nc.tensor.matmul(
                                    o_ps[:, lo:hi], vslice, a,
                                    start=st, stop=sp,
                                )

                    # natural j order: the first scores tiles touch only the
                    # already-roped first token chunk, so the head starts
                    # without waiting on the previous A-phase's rope tail.
                    # The batch-half's V transposes ride along in head 0.
                    acc(0, "l")
                    acc(0, "o")
                    if m == 0:
                        v_transpose(b * 8 + 4, b * 8 + 5)
                    scores(2)
                    acc(1, "l")
                    acc(1, "o")
                    if m == 0:
                        v_transpose(b * 8 + 5, b * 8 + 6)
                    scores(3)
                    acc(2, "l")
                    acc(2, "o")
                    if m == 0:
                        v_transpose(b * 8 + 6, b * 8 + 7)
                    scores(4)
                    acc(3, "l")
                    acc(3, "o")
                    norm_half(0)
                    if m == 0:
                        v_transpose(b * 8 + 7, b * 8 + 8)
                    scores(5)
                    acc(4, "l")
                    acc(4, "o")
                    scores(6)
                    acc(5, "l")
                    acc(5, "o")
                    scores(7)
                    acc(6, "l")
                    acc(6, "o")
                    acc(7, "l")
                    acc(7, "o")
                    norm_half(1)

            def all_gather(b):
                # Triggered from the gpsimd queue right after this batch's
                # attn_local writes; fires as soon as those DMAs land.
                for hp in range(2):
                    nc.gpsimd.collective_compute(
                        "AllGather",
                        ALU.bypass,
                        ins=[attn_local[b][hp][:, :]],
                        outs=[ag_out[b][hp][:, :]],
                        replica_groups=[list(range(NCORES))],
                    )

            # ---- phase D: out projection on this core's wo column shard ----
            # af chunks from the head-pair-0 gather come first so the phase
            # can start as soon as that (earlier-completing) collective
            # lands; pair-1 chunks follow.
            AF_ORDER = [af for af in range(HC) if af % 4 < 2] + \
                       [af for af in range(HC) if af % 4 >= 2]

            def phase_d(b):
                for pp in range(S // 512):
                    ops = [
                        psp.tile([128, 1024], f32, tag="big",
                                 name=f"op{b}_{pp}_{q}")
                        for q in range(2)
                    ]
                    for qi in range(0, HC, 4):
                        quad = AF_ORDER[qi:qi + 4]
                        ag_ts = []
                        for af in quad:
                            hp = (af % 4) // 2
                            r0 = (af // 4) * 256 + (af % 2) * 128
                            ag_t = agp.tile(
                                [128, 512], bf16, tag="ag", bufs=20,
                                name=f"ag{b}_{pp}_{af}"
                            )
                            eng2 = nc.sync if af % 2 == 0 else nc.scalar
                            eng2.dma_start(
                                ag_t[:],
                                ag_out[b][hp][r0:r0 + 128,
                                              pp * 512:(pp + 1) * 512],
                            )
                            ag_ts.append(ag_t)
                        # run-length-4 per psum bank
                        for tt in range(4):
                            for k, af in enumerate(quad):
                                nc.tensor.matmul(
                                    ops[tt // 2][:, (tt % 2) * 512:(tt % 2) * 512 + 512],
                                    ag_ts[k][:, tt * 128:(tt + 1) * 128],
                                    wo_sb[:, af * OUTC:(af + 1) * OUTC],
                                    start=(qi == 0 and k == 0),
                                    stop=(qi == HC - 4 and k == 3),
                                )
                    for q in range(2):
                        ob = miscp.tile([128, 1024], bf16, tag="ob",
                                        name=f"ob{b}_{pp}_{q}")
                        nc.vector.tensor_copy(ob[:], ops[q][:])
                        r0 = b * S + pp * 512 + q * 256
                        nc.sync.dma_start(out[r0:r0 + 128, :], ob[:, 0:512])
                        nc.sync.dma_start(out[r0 + 128:r0 + 256, :], ob[:, 512:1024])

            # ---- orchestration: batch-interleaved; every AllGather hides
            # under the other batch's projections / attention / out-proj ----
            phase_a(0, nc.scalar, pre0)
            load_split_wo_pieces = 16
            for p in range(load_split_wo_pieces):
                step = HC * OUTC // load_split_wo_pieces
                nc.scalar.dma_start(
                    wo_sb[:, p * step:(p + 1) * step],
                    wo[:, p * step:(p + 1) * step],
                )
            phase_a(1, nc.sync, trs=(0, 1, 2, 3))
            phase_b(0)
            phase_a(2, nc.sync)
            phase_a(3, nc.sync, trs=(8, 9, 10, 11))
            all_gather(0)           # fires right after B(b0) writes
            phase_b(1)
            # fence: keep the scheduler from hoisting AG0-dependent ag
            # loads above the A2/A3 hid loads on the sync/scalar engines
            tc.no_sync_barrier()
            phase_d(0)
            all_gather(1)           # fires right after B(b1) writes
            # fence: keep AG1-dependent D1 loads behind all D0 loads
            tc.no_sync_barrier()
            phase_d(1)

    nc.compile()
    return nc


@functools.lru_cache(maxsize=1)
def _get_nc():
    return _build()


def _prep_in_maps(hidden_states, wq, wk, wv, wo, cos, sin):
    hs = np.ascontiguousarray(np.asarray(hidden_states, np.float32)).reshape(T, HID)
    hidT = hs.T.reshape(HC, 128, T).transpose(1, 0, 2).reshape(128, HC * T)
    hidT = np.ascontiguousarray(hidT).astype(BF16)

    wq = np.asarray(wq, np.float32) * SCALE
    wk = np.asarray(wk, np.float32)
    wv = np.asarray(wv, np.float32)
    wo = np.asarray(wo, np.float32)

    cosT = np.ascontiguousarray(np.asarray(cos, np.float32).T)  # [64, S]
    sinT = np.ascontiguousarray(np.asarray(sin, np.float32).T)
    cosT2 = np.concatenate([cosT, cosT], axis=1)   # [64, T]
    sinT2 = np.concatenate([sinT, sinT], axis=1)
    cos128 = np.concatenate([cosT2, cosT2], axis=0)  # [128, T]
    sin128 = np.concatenate([-sinT2, sinT2], axis=0)

    r = np.arange(128)[:, None]
    c = np.arange(128)[None, :]
    SL01 = (c >= r).astype(np.float32)  # diag tile: valid where q >= k
    SU01 = (c <= r).astype(np.float32)  # edge tile: valid where q-k <= W
    m01 = np.concatenate([SL01, SU01], axis=1).astype(BF16)

    def shard_w(w, cols, core):
        ws = w[:, core * cols:(core + 1) * cols]
        return np.ascontiguousarray(
            ws.reshape(HC, 128, cols).transpose(1, 0, 2).reshape(128, HC * cols)
        ).astype(BF16)

    def shard_wo(w, core):
        # chunk ci of phase D reads rows a16=ci%16 of head-pair gather
        # p=ci//16, whose rows are (core c2=a16//2, head-in-pair hp=a16%2)
        # -> global head 4*c2 + 2*p + hp
        ws = w[:, core * OUTC:(core + 1) * OUTC]
        blocks = []
        for ci in range(HC):
            p, a16 = divmod(ci, 16)
            c2, hp = divmod(a16, 2)
            g = 4 * c2 + 2 * p + hp
            blocks.append(ws[g * 128:(g + 1) * 128, :])
        arr = np.stack(blocks, 0)
        return np.ascontiguousarray(
            arr.transpose(1, 0, 2).reshape(128, HC * OUTC)
        ).astype(BF16)

    in_maps = []
    for cidx in range(NCORES):
        in_maps.append({
            "hidT": hidT,
            "wq": shard_w(wq, QF, cidx),
            "wk": shard_w(wk, D, cidx),
            "wv": shard_w(wv, D, cidx),
            "wo": shard_w(wo, OUTC, cidx),
            "cosT": cos128,
            "sinT": sin128,
            "m01": m01,
        })
    return in_maps


def run(inputs, trace=False, **spmd_kwargs):
    from concourse.bass_utils import run_bass_kernel_spmd

    window = int(np.asarray(inputs["window"]))
    assert window == WINDOW, f"kernel compiled for window={WINDOW}, got {window}"
    nc = _get_nc()
    in_maps = _prep_in_maps(
        inputs["hidden_states"], inputs["wq"], inputs["wk"], inputs["wv"],
        inputs["wo"], inputs["cos"], inputs["sin"],
    )
    res = run_bass_kernel_spmd(
        nc, in_maps, list(range(NCORES)), trace=trace, **spmd_kwargs
    )
    parts = [
        np.asarray(res.results[i]["out"]).astype(np.float32)
        for i in range(NCORES)
    ]
    full = np.concatenate(parts, axis=1).reshape(B, S, HID)
    return full, res


def kernel(**inputs):
    return run(inputs, trace=False)[0]
